# revision 1
# baseline (speedup 1.0000x reference)
"""GroupedQueryAttention Trainium2 kernel (8 NeuronCores).

Sharding: core i handles (batch b = i//4, KV group g = i%4): its 4 query
heads + 1 KV group, full sequence. Each core computes a partial output
(attn_heads @ Wo rows for its heads); host sums the 4 partials per batch.

Layout strategy (per core):
  - everything transposed: qT/kT [d, t] computed with W-stationary matmuls
  - RoPE: host permutes W rows to half-split layout; swap-half via a
    permutation matmul on PE; cos/sin tables applied on DVE
  - attention: scoresT [s, tq] = kT_tile^T @ qT (stationary kT tile),
    exp on ACT (no max subtraction -- scores are bounded by construction),
    denominators via ones-vector matmul, PV with v[s,d]-stationary
    accumulation -> outT [d, tq], normalization by broadcast reciprocal.
  - out projection: attn_flatT chunks stationary, Wo.T moving.
All matmuls run as float32r (full-rate fp32 PE mode).
"""

import numpy as np
from contextlib import ExitStack

import concourse.bass as bass
import concourse.bacc as bacc
import concourse.tile as tile
import concourse.mybir as mybir
from concourse.bass_utils import run_bass_kernel_spmd

# problem shape (hardcoded per contract)
B, T, E = 2, 2048, 2048
NH, NG, HD = 16, 4, 128
HPG = NH // NG          # 4 heads per group = per core
NE = E // 128           # 16 contraction chunks
TB = 512                # tq / t block
NTB = T // TB           # 4
NST = T // 128          # 16 s-tiles
F32 = mybir.dt.float32
F32R = mybir.dt.float32r
EXP = mybir.ActivationFunctionType.Exp

N_CORES = 8


def _r(ap):
    return ap.bitcast(F32R)


def build_body(tc, out_ap, ins):
    """ins: dict name -> dram AP. out_ap: [T, E] dram AP."""
    nc = tc.nc
    ctx = ExitStack()
    with ctx:
        ctx.enter_context(nc.allow_low_precision(
            reason="fp32r rounding on matmul inputs is intended"))
        # ---- constant / persistent SBUF ----
        const = ctx.enter_context(tc.tile_pool(name="const", bufs=1))
        cs2 = const.tile([128, T], F32, tag="cs2", name="cs2")
        snpm = const.tile([128, T], F32, tag="snpm", name="snpm")
        tri = const.tile([128, 128], F32, tag="tri", name="tri")
        swp = const.tile([128, 128], F32R, tag="swp", name="swp")
        iden = const.tile([128, 128], F32, tag="iden", name="iden")
        ones = const.tile([128, 128], F32R, tag="ones", name="ones")
        zer = const.tile([128, TB], F32, tag="zer", name="zer")

        persist = ctx.enter_context(tc.tile_pool(name="persist", bufs=1))
        qrot = [persist.tile([128, T], F32, tag=f"qrot{h}", name=f"qrot{h}") for h in range(HPG)]
        krot = persist.tile([128, T], F32, tag="krot", name="krot")
        vsd = persist.tile([128, T], F32, tag="vsd", name="vsd")
        aout = qrot  # attn output overwrites qrot block-by-block (dead after scores)

        # ---- weights (packed into single wide tiles, col block = e-chunk) ----
        wpool = ctx.enter_context(tc.tile_pool(name="weights", bufs=1))
        wq_t = wpool.tile([128, NE * 512], F32R, tag="wbig", name="wq")    # block e: [128, 4*128]
        wk_t = wpool.tile([128, NE * 128], F32R, tag="wk", name="wk")
        wv_t = wpool.tile([128, NE * 128], F32R, tag="wv", name="wv")

        # ---- psum pools ----
        psp = ctx.enter_context(tc.tile_pool(name="psp", bufs=2, space="PSUM"))
        pssp = ctx.enter_context(tc.tile_pool(name="pssp", bufs=2, space="PSUM"))
        psop = ctx.enter_context(tc.tile_pool(name="psop", bufs=2, space="PSUM"))
        psdp = ctx.enter_context(tc.tile_pool(name="psdp", bufs=2, space="PSUM"))

        # ---- sbuf working pools ----
        xpool = ctx.enter_context(tc.tile_pool(name="xcol", bufs=20))
        qrpool = ctx.enter_context(tc.tile_pool(name="qraw", bufs=6))
        ptpool = ctx.enter_context(tc.tile_pool(name="pt", bufs=4))
        srpool = ctx.enter_context(tc.tile_pool(name="sr", bufs=4))
        ospool = ctx.enter_context(tc.tile_pool(name="osb", bufs=2))

        def rope(dst_ap, ps, cols):
            """dst = raw*cos + swap(raw)*sgn_sin, raw in psum ps [128, TB]."""
            qraw = qrpool.tile([128, TB], F32, tag="qraw", name="qraw")
            nc.scalar.copy(_r(qraw[:]), ps[:])
            ps_sw = pssp.tile([128, TB], F32, tag="pss", name="psw")
            nc.tensor.matmul(ps_sw[:], _r(swp[:]), _r(qraw[:]), start=True, stop=True)
            tmp1 = qrpool.tile([128, TB], F32, tag="qraw", name="ropetmp1")
            tmp2 = qrpool.tile([128, TB], F32, tag="qraw", name="ropetmp2")
            nc.vector.tensor_mul(tmp1[:], qraw[:], cs2[:, cols])
            nc.vector.tensor_mul(tmp2[:], ps_sw[:], snpm[:, cols])
            nc.vector.tensor_add(_r(dst_ap), tmp1[:], tmp2[:])

        # ================= projection phase =================
        for tb in range(NTB):
            cols = slice(tb * TB, (tb + 1) * TB)
            xc = []
            for e in range(NE):
                t_ = xpool.tile([128, TB], F32R, tag="xc", name="xc")
                nc.sync.dma_start(t_[:], _r(ins["xT"][e * 128:(e + 1) * 128, cols]))
                xc.append(t_)
            if tb == 0:
                # weights ordered so PE can start on k while q weights stream
                for e in range(NE):
                    r0 = e * 128
                    nc.sync.dma_start(wk_t[:, e * 128:(e + 1) * 128], _r(ins["wk"][r0:r0 + 128, :]))
                nc.sync.dma_start(swp[:], _r(ins["swp"][:]))
                for e in range(NE):
                    r0 = e * 128
                    nc.sync.dma_start(wv_t[:, e * 128:(e + 1) * 128], _r(ins["wv"][r0:r0 + 128, :]))
                nc.sync.dma_start(iden[:], ins["iden"][:])
                for e in range(NE):
                    r0 = e * 128
                    nc.sync.dma_start(wq_t[:, e * 512:(e + 1) * 512], _r(ins["wq"][r0:r0 + 128, :]))
                nc.sync.dma_start(cs2[:], ins["cs2"][:])
                nc.sync.dma_start(snpm[:], ins["snpm"][:])
                nc.sync.dma_start(tri[:], ins["tri"][:])
                nc.sync.dma_start(ones[:], _r(ins["onec"][:]))
                nc.sync.dma_start(zer[:], ins["zer"][:])

            ps_k = psp.tile([128, TB], F32, tag="ps", name="ps")
            for e in range(NE):
                nc.tensor.matmul(ps_k[:], _r(wk_t[:, e * 128:(e + 1) * 128]),
                                 _r(xc[e][:]), start=(e == 0), stop=(e == NE - 1))
            rope(krot[:, cols], ps_k, cols)

            ps_v = psp.tile([128, TB], F32, tag="ps", name="ps")
            for e in range(NE):
                nc.tensor.matmul(ps_v[:], _r(wv_t[:, e * 128:(e + 1) * 128]),
                                 _r(xc[e][:]), start=(e == 0), stop=(e == NE - 1))
            vtmp = qrpool.tile([128, TB], F32, tag="qraw", name="vtmp")
            nc.scalar.copy(vtmp[:], ps_v[:])

            # transpose v tiles of this block: vtmp [d, s] -> vsd [s, d]
            for jj in range(4):
                j = 4 * tb + jj
                pst = psp.tile([128, 128], F32, tag="ps", name="ps")
                nc.tensor.transpose(pst[:], vtmp[:, jj * 128:(jj + 1) * 128], iden[:])
                nc.scalar.copy(_r(vsd[:, j * 128:(j + 1) * 128]), pst[:])

            for dq in range(HPG):
                ps = psp.tile([128, TB], F32, tag="ps", name="ps")
                for e in range(NE):
                    nc.tensor.matmul(
                        ps[:],
                        _r(wq_t[:, e * 512 + dq * 128: e * 512 + (dq + 1) * 128]),
                        _r(xc[e][:]), start=(e == 0), stop=(e == NE - 1))
                rope(qrot[dq][:, cols], ps, cols)

        # wo: packed [128, 16*512], col block (hh*4+eo)
        wo_t = wpool.tile([128, NE * 512], F32R, tag="wbig", name="wo")
        for hh in range(HPG):
            for eo in range(4):
                blk = hh * 4 + eo
                nc.sync.dma_start(
                    wo_t[:, blk * 512:(blk + 1) * 512],
                    _r(ins["wo"][hh * 128:(hh + 1) * 128, eo * 512:(eo + 1) * 512]))

        # ================= attention phase =================
        for bi in range(NTB):
            for h in range(HPG):
                jmax = 4 * bi + 3
                pso = psop.tile([128, TB], F32, tag="pso", name="pso")
                psd = psdp.tile([128, TB], F32, tag="psd", name="psd")
                for j in range(jmax + 1):
                    diag = (j // 4 == bi)
                    o = 128 * (j - 4 * bi) if diag else 0
                    oe = min(o, 256)
                    W = TB - oe
                    pss = pssp.tile([128, TB], F32, tag="pss", name="pss")
                    nc.tensor.matmul(
                        pss[:, 0:W],
                        _r(krot[:, j * 128:(j + 1) * 128]),
                        _r(qrot[h][:, bi * TB + oe:(bi + 1) * TB]),
                        start=True, stop=True)
                    pt = ptpool.tile([128, TB], F32, tag="pt", name="pt")
                    nc.scalar.activation(_r(pt[:, oe:TB]), pss[:, 0:W], EXP)
                    if diag:
                        if o > 0:
                            nc.vector.tensor_copy(_r(pt[:, 0:o]), zer[:, 0:o])
                        nc.vector.tensor_mul(_r(pt[:, o:o + 128]), pt[:, o:o + 128], tri[:])
                    nc.tensor.matmul(psd[:], _r(ones[:]), _r(pt[:]),
                                     start=(j == 0), stop=(j == jmax))
                    nc.tensor.matmul(pso[:], _r(vsd[:, j * 128:(j + 1) * 128]),
                                     _r(pt[:]), start=(j == 0), stop=(j == jmax))
                cols = slice(bi * TB, (bi + 1) * TB)
                rden = srpool.tile([128, TB], F32, tag="rden", name="rden")
                nc.vector.reciprocal_approx_fast(rden[:], psd[:])
                nc.vector.tensor_mul(_r(aout[h][:, cols]), pso[:], rden[:])

        # ================= output projection =================
        for tq in range(NST):
            trows = slice(tq * 128, (tq + 1) * 128)
            for half in range(2):
                poa = pssp.tile([128, TB], F32, tag="pss", name="pss")
                pob = psop.tile([128, TB], F32, tag="pso", name="pso")
                for hh in range(HPG):
                    lh = _r(aout[hh][:, trows])
                    ba = hh * 4 + 2 * half
                    nc.tensor.matmul(poa[:], lh, _r(wo_t[:, ba * 512:(ba + 1) * 512]),
                                     start=(hh == 0), stop=(hh == HPG - 1))
                    nc.tensor.matmul(pob[:], lh, _r(wo_t[:, (ba + 1) * 512:(ba + 2) * 512]),
                                     start=(hh == 0), stop=(hh == HPG - 1))
                for k, po in ((0, poa), (1, pob)):
                    eo = 2 * half + k
                    osb = ospool.tile([128, TB], F32, tag="osb", name="osb")
                    nc.scalar.copy(osb[:], po[:])
                    nc.sync.dma_start(out_ap[trows, eo * 512:(eo + 1) * 512], osb[:])


# ---------------- host side ----------------

_PERM = np.concatenate([np.arange(0, HD, 2), np.arange(1, HD, 2)])  # half-split


def host_prep(inputs):
    """Full inputs -> list of 8 per-core input dicts (core i = (b=i//4, g=i%4))."""
    x = np.asarray(inputs["x"], dtype=np.float32)
    Wq = np.asarray(inputs["Wq"], dtype=np.float32)
    Wk = np.asarray(inputs["Wk"], dtype=np.float32)
    Wv = np.asarray(inputs["Wv"], dtype=np.float32)
    Wo = np.asarray(inputs["Wo"], dtype=np.float32)

    inv = (10000.0 ** (-np.arange(0, HD, 2, dtype=np.float32) / HD)).astype(np.float32)
    tpos = np.arange(T, dtype=np.float32)
    fr = np.outer(tpos, inv)                       # [T, 64]
    cosT = np.cos(fr).T.astype(np.float32)         # [64, T]
    sinT = np.sin(fr).T.astype(np.float32)
    cs2 = np.concatenate([cosT, cosT], axis=0)     # [128, T]
    snpm = np.concatenate([-sinT, sinT], axis=0)   # [128, T]

    tri = (np.arange(128)[None, :] >= np.arange(128)[:, None]).astype(np.float32)
    swp = np.zeros((128, 128), dtype=np.float32)
    swp[(np.arange(128) + 64) % 128, np.arange(128)] = 1.0
    iden = np.eye(128, dtype=np.float32)

    scale = np.float32(1.0 / np.sqrt(HD))
    xT = [np.ascontiguousarray(x[b].T) for b in range(B)]

    in_maps = []
    for i in range(N_CORES):
        b, g = i // 4, i % 4
        # wq: rows for heads g*4..g*4+3, each permuted, scaled; -> [E, 512]
        rows = []
        for h in range(HPG):
            base = (g * HPG + h) * HD
            rows.append(Wq[base + _PERM, :])
        wq_c = (np.concatenate(rows, axis=0) * scale).T  # [E, 512]
        wk_c = Wk[g * HD + _PERM, :].T                   # [E, 128]
        wv_c = Wv[g * HD:(g + 1) * HD, :].T              # [E, 128]
        wo_c = np.ascontiguousarray(Wo[:, g * 512:(g + 1) * 512].T)  # [512, E]
        in_maps.append({
            "xT": xT[b],
            "wq": np.ascontiguousarray(wq_c),
            "wk": np.ascontiguousarray(wk_c),
            "wv": np.ascontiguousarray(wv_c),
            "wo": wo_c,
            "cs2": cs2, "snpm": snpm, "tri": tri, "swp": swp, "iden": iden,
            "onec": np.ones((128, 128), dtype=np.float32),
            "zer": np.zeros((128, TB), dtype=np.float32),
        })
    return in_maps


_NC = None


def build_nc():
    global _NC
    if _NC is not None:
        return _NC
    nc = bacc.Bacc("TRN2", target_bir_lowering=False, debug=False,
                   num_devices=N_CORES)
    ins = {
        "xT": nc.dram_tensor("xT", [E, T], F32R, kind="ExternalInput").ap(),
        "wq": nc.dram_tensor("wq", [E, HPG * HD], F32R, kind="ExternalInput").ap(),
        "wk": nc.dram_tensor("wk", [E, HD], F32R, kind="ExternalInput").ap(),
        "wv": nc.dram_tensor("wv", [E, HD], F32R, kind="ExternalInput").ap(),
        "wo": nc.dram_tensor("wo", [HPG * HD, E], F32R, kind="ExternalInput").ap(),
        "cs2": nc.dram_tensor("cs2", [128, T], F32, kind="ExternalInput").ap(),
        "snpm": nc.dram_tensor("snpm", [128, T], F32, kind="ExternalInput").ap(),
        "tri": nc.dram_tensor("tri", [128, 128], F32, kind="ExternalInput").ap(),
        "swp": nc.dram_tensor("swp", [128, 128], F32R, kind="ExternalInput").ap(),
        "iden": nc.dram_tensor("iden", [128, 128], F32, kind="ExternalInput").ap(),
        "onec": nc.dram_tensor("onec", [128, 128], F32R, kind="ExternalInput").ap(),
        "zer": nc.dram_tensor("zer", [128, TB], F32, kind="ExternalInput").ap(),
    }
    out = nc.dram_tensor("out", [T, E], F32, kind="ExternalOutput").ap()
    with tile.TileContext(nc) as tc:
        build_body(tc, out, ins)
    nc.compile()
    _NC = nc
    return nc


def gather(results):
    """results: list of 8 dicts with 'out' [T, E] partials -> [B, T, E]."""
    out = np.zeros((B, T, E), dtype=np.float32)
    for i in range(N_CORES):
        out[i // 4] += results[i]["out"]
    return out


def kernel(**inputs):
    nc = build_nc()
    in_maps = host_prep(inputs)
    res = run_bass_kernel_spmd(nc, in_maps, core_ids=list(range(N_CORES)))
    return gather(res.results)


if __name__ == "__main__":
    rng = np.random.default_rng(0)
    ins = {
        "x": rng.standard_normal((B, T, E), dtype=np.float32),
        "Wq": rng.standard_normal((E, E), dtype=np.float32) * 0.02,
        "Wk": rng.standard_normal((NG * HD, E), dtype=np.float32) * 0.02,
        "Wv": rng.standard_normal((NG * HD, E), dtype=np.float32) * 0.02,
        "Wo": rng.standard_normal((E, E), dtype=np.float32) * 0.02,
    }
    out = kernel(**ins)
    print(out.shape, out.dtype, np.abs(out).mean())



# revision 2
# speedup vs baseline: 1.5202x; 1.5202x over previous
"""GroupedQueryAttention Trainium2 kernel (8 NeuronCores).

Sharding: core i handles (batch b = i//4, KV group g = i%4): its 4 query
heads + 1 KV group, full sequence. Each core computes a partial output
(attn_heads @ Wo rows for its heads); host sums the 4 partials per batch.

v2 layout strategy (per core), all matmul operands bf16 (fp32 PSUM):
  - x resident in SBUF as [128, e-chunk, t] bf16; weights pre-arranged on
    host into exact SBUF layouts so each loads with ONE contiguous DMA.
  - projections W-stationary: qT/kT [d, t] accumulated over 16 e-chunks.
  - RoPE: host permutes W rows to half-split layout; swap-half via a
    permutation matmul on PE; cos/sin tables applied on DVE (bf16 2x).
  - attention interleaved with projections (attention block bi runs right
    after projection block tb=bi): scoresT [s, tq] = kT_tile^T @ qT,
    exp on ACT (scores bounded by construction; no max subtraction),
    denominators via ones-matmul, PV with v[s,d]-stationary accumulation
    -> outT [d, tq]; diagonal blocks narrowed to the causal width.
  - out projection: attn tiles stationary, Wo chunks moving; results
    copied to bf16 SBUF (DVE/ACT alternating) and DMAd per 128-row stripe.
"""

import numpy as np
import ml_dtypes
from contextlib import ExitStack

import concourse.bass as bass
import concourse.bacc as bacc
import concourse.tile as tile
import concourse.mybir as mybir
from concourse.bass_utils import run_bass_kernel_spmd

# problem shape (hardcoded per contract)
B, T, E = 2, 2048, 2048
NH, NG, HD = 16, 4, 128
HPG = NH // NG          # 4 heads per group = per core
NE = E // 128           # 16 contraction chunks
TB = 512                # tq / t block
NTB = T // TB           # 4
NST = T // 128          # 16 t-tiles
F32 = mybir.dt.float32
BF16 = mybir.dt.bfloat16
EXP = mybir.ActivationFunctionType.Exp

N_CORES = 8
BF = ml_dtypes.bfloat16


def build_body(tc, out_ap, ins):
    """ins: dict name -> dram AP. out_ap: [T, E] bf16 dram AP."""
    nc = tc.nc
    ctx = ExitStack()
    with ctx:
        ctx.enter_context(nc.allow_low_precision(
            reason="bf16 matmul inputs / bf16 intermediate rounding is intended"))
        # ---- constant / persistent SBUF ----
        const = ctx.enter_context(tc.tile_pool(name="const", bufs=1))
        cs2 = const.tile([128, T], BF16, tag="cs2", name="cs2")
        snpm = const.tile([128, T], BF16, tag="snpm", name="snpm")
        tri = const.tile([128, 128], BF16, tag="tri", name="tri")
        swp = const.tile([128, 128], BF16, tag="swp", name="swp")
        iden = const.tile([128, 128], BF16, tag="iden", name="iden")
        ones = const.tile([128, 128], BF16, tag="ones", name="ones")

        persist = ctx.enter_context(tc.tile_pool(name="persist", bufs=1))
        xblk = [persist.tile([128, NE * TB], BF16, tag=f"xb{t}", name=f"xb{t}")
                for t in range(NTB)]
        qrot = [persist.tile([128, T], BF16, tag=f"qrot{h}", name=f"qrot{h}")
                for h in range(HPG)]
        aout = [persist.tile([128, T], BF16, tag=f"aout{h}", name=f"aout{h}")
                for h in range(HPG)]
        krot = persist.tile([128, T], BF16, tag="krot", name="krot")
        vsd = persist.tile([128, T], BF16, tag="vsd", name="vsd")

        # ---- weights (packed into single wide tiles, col block = e-chunk) ----
        wpool = ctx.enter_context(tc.tile_pool(name="weights", bufs=1))
        wq_t = wpool.tile([128, NE * 512], BF16, tag="wq", name="wq")
        wk_t = wpool.tile([128, NE * 128], BF16, tag="wk", name="wk")
        wv_t = wpool.tile([128, NE * 128], BF16, tag="wv", name="wv")
        wo_t = wpool.tile([128, NE * 512], BF16, tag="wo", name="wo")

        # ---- psum pools (8 banks total) ----
        psp = ctx.enter_context(tc.tile_pool(name="psp", bufs=2, space="PSUM"))
        pssp = ctx.enter_context(tc.tile_pool(name="pssp", bufs=2, space="PSUM"))
        psop = ctx.enter_context(tc.tile_pool(name="psop", bufs=2, space="PSUM"))
        psdp = ctx.enter_context(tc.tile_pool(name="psdp", bufs=2, space="PSUM"))

        # ---- sbuf working pools ----
        qrpool = ctx.enter_context(tc.tile_pool(name="qraw", bufs=6))
        ptpool = ctx.enter_context(tc.tile_pool(name="pt", bufs=6))
        srpool = ctx.enter_context(tc.tile_pool(name="sr", bufs=4))
        ospool = ctx.enter_context(tc.tile_pool(name="osb", bufs=3))

        # ---- input DMAs (few, large, host-prearranged layouts) ----
        xh3 = ins["xh"].rearrange("p (e t) -> p e t", t=T)

        def load_xblk(t):
            dst = xblk[t][:].rearrange("p (e c) -> p e c", c=TB)
            nc.sync.dma_start(dst, xh3[:, :, t * TB:(t + 1) * TB])

        nc.sync.dma_start(wk_t[:], ins["wk"][:])
        nc.sync.dma_start(swp[:], ins["swp"][:])
        nc.sync.dma_start(cs2[:], ins["cs2"][:])
        nc.sync.dma_start(snpm[:], ins["snpm"][:])
        load_xblk(0)
        nc.sync.dma_start(wv_t[:], ins["wv"][:])
        nc.sync.dma_start(iden[:], ins["iden"][:])
        nc.sync.dma_start(tri[:], ins["tri"][:])
        nc.sync.dma_start(ones[:], ins["onec"][:])
        load_xblk(1)
        nc.sync.dma_start(wq_t[:], ins["wq"][:])
        load_xblk(2)
        load_xblk(3)

        def rope(dst_ap, ps, cols):
            """dst = raw*cos + swap(raw)*sgn_sin, raw in psum ps [128, TB]."""
            qraw = qrpool.tile([128, TB], BF16, tag="qraw", name="qraw")
            nc.scalar.copy(qraw[:], ps[:])
            ps_sw = psp.tile([128, TB], F32, tag="ps", name="psw")
            nc.tensor.matmul(ps_sw[:], swp[:], qraw[:], start=True, stop=True)
            tmp1 = qrpool.tile([128, TB], BF16, tag="qraw", name="ropetmp1")
            nc.vector.tensor_mul(tmp1[:], qraw[:], cs2[:, cols])
            tmp2 = qrpool.tile([128, TB], BF16, tag="qraw", name="ropetmp2")
            nc.vector.tensor_mul(tmp2[:], ps_sw[:], snpm[:, cols])
            nc.vector.tensor_add(dst_ap, tmp1[:], tmp2[:])

        def project_block(tb):
            cols = slice(tb * TB, (tb + 1) * TB)
            xe = lambda e: xblk[tb][:, e * TB:(e + 1) * TB]

            ps_k = psp.tile([128, TB], F32, tag="ps", name="ps")
            for e in range(NE):
                nc.tensor.matmul(ps_k[:], wk_t[:, e * 128:(e + 1) * 128],
                                 xe(e), start=(e == 0), stop=(e == NE - 1))
            rope(krot[:, cols], ps_k, cols)

            ps_v = psp.tile([128, TB], F32, tag="ps", name="ps")
            for e in range(NE):
                nc.tensor.matmul(ps_v[:], wv_t[:, e * 128:(e + 1) * 128],
                                 xe(e), start=(e == 0), stop=(e == NE - 1))
            vtmp = qrpool.tile([128, TB], BF16, tag="qraw", name="vtmp")
            nc.scalar.copy(vtmp[:], ps_v[:])

            # transpose v tiles of this block: vtmp [d, s] -> vsd [s, d]
            for jj in range(4):
                j = 4 * tb + jj
                pst = psp.tile([128, 128], BF16, tag="ps", name="pst")
                nc.tensor.transpose(pst[:], vtmp[:, jj * 128:(jj + 1) * 128], iden[:])
                nc.vector.tensor_copy(vsd[:, j * 128:(j + 1) * 128], pst[:])

            for dq in range(HPG):
                ps = psp.tile([128, TB], F32, tag="ps", name="ps")
                for e in range(NE):
                    nc.tensor.matmul(
                        ps[:],
                        wq_t[:, e * 512 + dq * 128: e * 512 + (dq + 1) * 128],
                        xe(e), start=(e == 0), stop=(e == NE - 1))
                rope(qrot[dq][:, cols], ps, cols)

        def attention_block(bi):
            jmax = 4 * bi + 3
            for h in range(HPG):
                pso = psop.tile([128, TB], F32, tag="pso", name="pso")
                psd = psdp.tile([128, TB], F32, tag="psd", name="psd")
                for j in range(jmax + 1):
                    diag = (j // 4 == bi)
                    o = 128 * (j - 4 * bi) if diag else 0
                    W = TB - o
                    pool = pssp if (j % 2 == 0) else psp
                    pss = pool.tile([128, TB], F32, tag=pool is psp and "ps" or "pss",
                                    name="pss")
                    nc.tensor.matmul(
                        pss[:, 0:W],
                        krot[:, j * 128:(j + 1) * 128],
                        qrot[h][:, bi * TB + o:(bi + 1) * TB],
                        start=True, stop=True)
                    pt = ptpool.tile([128, TB], BF16, tag="pt", name="pt")
                    nc.scalar.activation(pt[:, o:TB], pss[:, 0:W], EXP)
                    if diag and o < TB - 128 + 1:
                        nc.vector.tensor_mul(pt[:, o:o + 128], pt[:, o:o + 128], tri[:])
                    nc.tensor.matmul(psd[:, o:TB], ones[:], pt[:, o:TB],
                                     start=(j == 0), stop=(j == jmax))
                    nc.tensor.matmul(pso[:, o:TB], vsd[:, j * 128:(j + 1) * 128],
                                     pt[:, o:TB], start=(j == 0), stop=(j == jmax))
                cols = slice(bi * TB, (bi + 1) * TB)
                rden = srpool.tile([128, TB], F32, tag="rden", name="rden")
                nc.vector.reciprocal_approx_fast(rden[:], psd[:])
                nc.vector.tensor_mul(aout[h][:, cols], pso[:], rden[:])

        # ======== fused projection + attention (per 512-token block) ========
        for tb in range(NTB):
            project_block(tb)
            if tb == NTB - 1:
                nc.sync.dma_start(wo_t[:], ins["wo"][:])
            attention_block(tb)

        # ================= output projection =================
        opools = [psp, pssp, psop, psdp]
        otags = ["ps", "pss", "pso", "psd"]
        for tq in range(NST):
            trows = slice(tq * 128, (tq + 1) * 128)
            pos = [opools[k].tile([128, TB], F32, tag=otags[k], name="po")
                   for k in range(4)]
            for hh in range(HPG):
                lh = aout[hh][:, trows]
                for eo in range(4):
                    nc.tensor.matmul(pos[eo][:],
                                     lh, wo_t[:, (hh * 4 + eo) * 512:(hh * 4 + eo + 1) * 512],
                                     start=(hh == 0), stop=(hh == HPG - 1))
            osb = ospool.tile([128, 4 * TB], BF16, tag="osb", name="osb")
            for eo in range(4):
                eng = nc.vector.tensor_copy if eo % 2 == 0 else nc.scalar.copy
                eng(osb[:, eo * TB:(eo + 1) * TB], pos[eo][:])
            nc.sync.dma_start(out_ap[trows, :], osb[:])


# ---------------- host side ----------------

_PERM = np.concatenate([np.arange(0, HD, 2), np.arange(1, HD, 2)])  # half-split


def _chunked(a, ncols):
    """[E, ncols] -> [128, NE*ncols] with col block e = rows e*128:(e+1)*128."""
    return np.ascontiguousarray(
        a.reshape(NE, 128, ncols).transpose(1, 0, 2).reshape(128, NE * ncols))


def host_prep(inputs):
    """Full inputs -> list of 8 per-core input dicts (core i = (b=i//4, g=i%4))."""
    x = np.asarray(inputs["x"], dtype=np.float32)
    Wq = np.asarray(inputs["Wq"], dtype=np.float32)
    Wk = np.asarray(inputs["Wk"], dtype=np.float32)
    Wv = np.asarray(inputs["Wv"], dtype=np.float32)
    Wo = np.asarray(inputs["Wo"], dtype=np.float32)

    inv = (10000.0 ** (-np.arange(0, HD, 2, dtype=np.float32) / HD)).astype(np.float32)
    tpos = np.arange(T, dtype=np.float32)
    fr = np.outer(tpos, inv)                       # [T, 64]
    cosT = np.cos(fr).T.astype(np.float32)         # [64, T]
    sinT = np.sin(fr).T.astype(np.float32)
    cs2 = np.concatenate([cosT, cosT], axis=0).astype(BF)     # [128, T]
    snpm = np.concatenate([-sinT, sinT], axis=0).astype(BF)   # [128, T]

    tri = (np.arange(128)[None, :] >= np.arange(128)[:, None]).astype(BF)
    swp = np.zeros((128, 128), dtype=np.float32)
    swp[(np.arange(128) + 64) % 128, np.arange(128)] = 1.0
    swp = swp.astype(BF)
    iden = np.eye(128, dtype=np.float32).astype(BF)

    scale = np.float32(1.0 / np.sqrt(HD))
    # x[b].T chunked: xh[p, e*T + t] = x[b][t, e*128+p]
    xh = [_chunked(np.ascontiguousarray(x[b].T), T).astype(BF) for b in range(B)]

    in_maps = []
    for i in range(N_CORES):
        b, g = i // 4, i % 4
        # wq: rows for heads g*4..g*4+3, each permuted, scaled; -> [E, 512]
        rows = []
        for h in range(HPG):
            base = (g * HPG + h) * HD
            rows.append(Wq[base + _PERM, :])
        wq_c = (np.concatenate(rows, axis=0) * scale).T  # [E, 512]
        wk_c = Wk[g * HD + _PERM, :].T                   # [E, 128]
        wv_c = Wv[g * HD:(g + 1) * HD, :].T              # [E, 128]
        wo_c = np.ascontiguousarray(Wo[:, g * 512:(g + 1) * 512].T)  # [512, E]
        # device layout [128, 16*512]: col block (hh*4+eo) = wo_c[hh*128:.., eo*512:..]
        wo_p = np.ascontiguousarray(
            wo_c.reshape(HPG, 128, 4, 512).transpose(1, 0, 2, 3).reshape(128, NE * 512))
        in_maps.append({
            "xh": xh[b],
            "wq": _chunked(wq_c, 512).astype(BF),
            "wk": _chunked(wk_c, 128).astype(BF),
            "wv": _chunked(wv_c, 128).astype(BF),
            "wo": wo_p.astype(BF),
            "cs2": cs2, "snpm": snpm, "tri": tri, "swp": swp, "iden": iden,
            "onec": np.ones((128, 128), dtype=BF),
        })
    return in_maps


_NC = None


def build_nc():
    global _NC
    if _NC is not None:
        return _NC
    nc = bacc.Bacc("TRN2", target_bir_lowering=False, debug=False,
                   num_devices=N_CORES)
    ins = {
        "xh": nc.dram_tensor("xh", [128, NE * T], BF16, kind="ExternalInput").ap(),
        "wq": nc.dram_tensor("wq", [128, NE * 512], BF16, kind="ExternalInput").ap(),
        "wk": nc.dram_tensor("wk", [128, NE * 128], BF16, kind="ExternalInput").ap(),
        "wv": nc.dram_tensor("wv", [128, NE * 128], BF16, kind="ExternalInput").ap(),
        "wo": nc.dram_tensor("wo", [128, NE * 512], BF16, kind="ExternalInput").ap(),
        "cs2": nc.dram_tensor("cs2", [128, T], BF16, kind="ExternalInput").ap(),
        "snpm": nc.dram_tensor("snpm", [128, T], BF16, kind="ExternalInput").ap(),
        "tri": nc.dram_tensor("tri", [128, 128], BF16, kind="ExternalInput").ap(),
        "swp": nc.dram_tensor("swp", [128, 128], BF16, kind="ExternalInput").ap(),
        "iden": nc.dram_tensor("iden", [128, 128], BF16, kind="ExternalInput").ap(),
        "onec": nc.dram_tensor("onec", [128, 128], BF16, kind="ExternalInput").ap(),
    }
    out = nc.dram_tensor("out", [T, E], BF16, kind="ExternalOutput").ap()
    with tile.TileContext(nc) as tc:
        build_body(tc, out, ins)
    nc.compile()
    _NC = nc
    return nc


def gather(results):
    """results: list of 8 dicts with 'out' [T, E] bf16 partials -> [B, T, E] f32."""
    out = np.zeros((B, T, E), dtype=np.float32)
    for i in range(N_CORES):
        out[i // 4] += np.asarray(results[i]["out"]).astype(np.float32)
    return out


def kernel(**inputs):
    nc = build_nc()
    in_maps = host_prep(inputs)
    res = run_bass_kernel_spmd(nc, in_maps, core_ids=list(range(N_CORES)))
    return gather(res.results)


if __name__ == "__main__":
    rng = np.random.default_rng(0)
    ins = {
        "x": rng.standard_normal((B, T, E), dtype=np.float32),
        "Wq": rng.standard_normal((E, E), dtype=np.float32) * 0.02,
        "Wk": rng.standard_normal((NG * HD, E), dtype=np.float32) * 0.02,
        "Wv": rng.standard_normal((NG * HD, E), dtype=np.float32) * 0.02,
        "Wo": rng.standard_normal((E, E), dtype=np.float32) * 0.02,
    }
    out = kernel(**ins)
    print(out.shape, out.dtype, np.abs(out).mean())


# revision 7
# speedup vs baseline: 1.6027x; 1.0542x over previous
"""GroupedQueryAttention Trainium2 kernel (8 NeuronCores).

Sharding: core i handles (batch b = i//4, KV group g = i%4): its 4 query
heads + 1 KV group, full sequence. Each core computes a partial output
(attn_heads @ Wo rows for its heads); host sums the 4 partials per batch.

v2 layout strategy (per core), all matmul operands bf16 (fp32 PSUM):
  - x resident in SBUF as [128, e-chunk, t] bf16; weights pre-arranged on
    host into exact SBUF layouts so each loads with ONE contiguous DMA.
  - projections W-stationary: qT/kT [d, t] accumulated over 16 e-chunks.
  - RoPE: host permutes W rows to half-split layout; swap-half via a
    permutation matmul on PE; cos/sin tables applied on DVE (bf16 2x).
  - attention interleaved with projections (attention block bi runs right
    after projection block tb=bi): scoresT [s, tq] = kT_tile^T @ qT,
    exp on ACT (scores bounded by construction; no max subtraction),
    denominators via ones-matmul, PV with v[s,d]-stationary accumulation
    -> outT [d, tq]; diagonal blocks narrowed to the causal width.
  - out projection: attn tiles stationary, Wo chunks moving; results
    copied to bf16 SBUF (DVE/ACT alternating) and DMAd per 128-row stripe.
"""

import numpy as np
import ml_dtypes
from contextlib import ExitStack

import concourse.bass as bass
import concourse.bacc as bacc
import concourse.tile as tile
import concourse.mybir as mybir
from concourse.bass_utils import run_bass_kernel_spmd

# problem shape (hardcoded per contract)
B, T, E = 2, 2048, 2048
NH, NG, HD = 16, 4, 128
HPG = NH // NG          # 4 heads per group = per core
NE = E // 128           # 16 contraction chunks
TB = 512                # tq / t block
NTB = T // TB           # 4
NST = T // 128          # 16 t-tiles
F32 = mybir.dt.float32
BF16 = mybir.dt.bfloat16
EXP = mybir.ActivationFunctionType.Exp

N_CORES = 8
BF = ml_dtypes.bfloat16


def build_body(tc, out_ap, ins):
    """ins: dict name -> dram AP. out_ap: [T, E] bf16 dram AP."""
    nc = tc.nc
    ctx = ExitStack()
    with ctx:
        ctx.enter_context(nc.allow_low_precision(
            reason="bf16 matmul inputs / bf16 intermediate rounding is intended"))
        # ---- constant / persistent SBUF ----
        const = ctx.enter_context(tc.tile_pool(name="const", bufs=1))
        cs2 = const.tile([128, T], BF16, tag="cs2", name="cs2")
        snpm = const.tile([128, T], BF16, tag="snpm", name="snpm")
        tri = const.tile([128, 128], BF16, tag="tri", name="tri")
        swp = const.tile([128, 128], BF16, tag="swp", name="swp")
        iden = const.tile([128, 128], BF16, tag="iden", name="iden")
        ones = const.tile([128, 128], BF16, tag="ones", name="ones")

        persist = ctx.enter_context(tc.tile_pool(name="persist", bufs=1))
        xblk = [persist.tile([128, NE * TB], BF16, tag=f"xb{t}", name=f"xb{t}")
                for t in range(NTB)]
        qrot = [persist.tile([128, T], BF16, tag=f"qrot{h}", name=f"qrot{h}")
                for h in range(HPG)]
        aout = [persist.tile([128, T], BF16, tag=f"aout{h}", name=f"aout{h}")
                for h in range(HPG)]
        krot = persist.tile([128, T], BF16, tag="krot", name="krot")
        vsd = persist.tile([128, T], BF16, tag="vsd", name="vsd")

        # ---- weights (packed into single wide tiles, col block = e-chunk) ----
        wpool = ctx.enter_context(tc.tile_pool(name="weights", bufs=1))
        wq_t = wpool.tile([128, NE * 512], BF16, tag="wq", name="wq")
        wk_t = wpool.tile([128, NE * 128], BF16, tag="wk", name="wk")
        wv_t = wpool.tile([128, NE * 128], BF16, tag="wv", name="wv")
        wo_t = wpool.tile([128, NE * 512], BF16, tag="wo", name="wo")

        # ---- psum pools: 2 pools x 2 bufs x 2-bank slots = 8 banks ----
        # P1: projection accumulators + attention score-pairs + outproj eo01
        # P2: rope-swap / v-transpose scratch + attention (pso|psd) + outproj eo23
        P1 = ctx.enter_context(tc.tile_pool(name="P1", bufs=2, space="PSUM"))
        P2 = ctx.enter_context(tc.tile_pool(name="P2", bufs=2, space="PSUM"))

        # ---- sbuf working pools ----
        qrpool = ctx.enter_context(tc.tile_pool(name="qraw", bufs=6))
        ptpool = ctx.enter_context(tc.tile_pool(name="pt", bufs=6))
        srpool = ctx.enter_context(tc.tile_pool(name="sr", bufs=4))
        ospool = ctx.enter_context(tc.tile_pool(name="osb", bufs=3))

        # ---- input DMAs (few, large, host-prearranged layouts) ----
        # Two HW queues: big streams on sync(SP), constants on scalar(ACT).
        xh3 = ins["xh"].rearrange("p (e t) -> p e t", t=T)

        def load_xblk(t, splits=1):
            dst = xblk[t][:].rearrange("p (e c) -> p e c", c=TB)
            step = NE // splits
            for s in range(splits):
                es = slice(s * step, (s + 1) * step)
                nc.sync.dma_start(dst[:, es, :],
                                  xh3[:, es, t * TB:(t + 1) * TB])

        nc.sync.dma_start(wk_t[:], ins["wk"][:])
        nc.scalar.dma_start(swp[:], ins["swp"][:])
        nc.scalar.dma_start(wv_t[:], ins["wv"][:])
        load_xblk(0, splits=4)
        nc.scalar.dma_start(iden[:], ins["iden"][:])
        nc.scalar.dma_start(cs2[:], ins["cs2"][:])
        nc.scalar.dma_start(snpm[:], ins["snpm"][:])
        nc.scalar.dma_start(tri[:], ins["tri"][:])
        nc.scalar.dma_start(ones[:], ins["onec"][:])
        for s in range(4):  # wq quarters by e-chunk so q-proj starts early
            cols4 = slice(s * 4 * 512, (s + 1) * 4 * 512)
            nc.sync.dma_start(wq_t[:, cols4], ins["wq"][:, cols4])
        load_xblk(1, splits=2)
        load_xblk(2)
        load_xblk(3)

        # pending PE work (part-2 of rope / v-transpose), emitted after the
        # NEXT accumulation group's matmuls so the ACT psum->sbuf copy
        # latency hides under queued PE work.
        pend = []

        def flush_pend():
            while pend:
                pend.pop(0)()

        def rope(dst_ap, ps, cols):
            """dst = raw*cos + swap(raw)*sgn_sin, raw in psum ps [128, TB]."""
            qraw = qrpool.tile([128, TB], BF16, tag="qraw", name="qraw")
            nc.scalar.copy(qraw[:], ps[:])

            def part2():
                ps_sw = P2.tile([128, TB], F32, tag="acc", name="psw")
                nc.tensor.matmul(ps_sw[:], swp[:], qraw[:], start=True, stop=True)
                tmp1 = qrpool.tile([128, TB], BF16, tag="qraw", name="ropetmp1")
                nc.vector.tensor_mul(tmp1[:], qraw[:], cs2[:, cols])
                tmp2 = qrpool.tile([128, TB], BF16, tag="qraw", name="ropetmp2")
                nc.vector.tensor_mul(tmp2[:], ps_sw[:], snpm[:, cols])
                nc.vector.tensor_add(dst_ap, tmp1[:], tmp2[:])
            pend.append(part2)

        def project_block(tb):
            cols = slice(tb * TB, (tb + 1) * TB)
            xe = lambda e: xblk[tb][:, e * TB:(e + 1) * TB]

            ps_k = P1.tile([128, TB], F32, tag="ps", name="ps")
            for e in range(NE):
                nc.tensor.matmul(ps_k[:], wk_t[:, e * 128:(e + 1) * 128],
                                 xe(e), start=(e == 0), stop=(e == NE - 1))
            flush_pend()
            rope(krot[:, cols], ps_k, cols)

            ps_v = P1.tile([128, TB], F32, tag="ps", name="ps")
            for e in range(NE):
                nc.tensor.matmul(ps_v[:], wv_t[:, e * 128:(e + 1) * 128],
                                 xe(e), start=(e == 0), stop=(e == NE - 1))
            flush_pend()
            vtmp = qrpool.tile([128, TB], BF16, tag="qraw", name="vtmp")
            nc.scalar.copy(vtmp[:], ps_v[:])

            def vtrans():
                # transpose v tiles of this block: vtmp [d, s] -> vsd [s, d]
                for jj in range(4):
                    j = 4 * tb + jj
                    pst = P2.tile([128, 128], BF16, tag="acc", name="pst")
                    nc.tensor.transpose(pst[:], vtmp[:, jj * 128:(jj + 1) * 128],
                                        iden[:])
                    nc.vector.tensor_copy(vsd[:, j * 128:(j + 1) * 128], pst[:])
            pend.append(vtrans)

            for dq in range(HPG):
                ps = P1.tile([128, TB], F32, tag="ps", name="ps")
                for e in range(NE):
                    nc.tensor.matmul(
                        ps[:],
                        wq_t[:, e * 512 + dq * 128: e * 512 + (dq + 1) * 128],
                        xe(e), start=(e == 0), stop=(e == NE - 1))
                flush_pend()
                rope(qrot[dq][:, cols], ps, cols)

        def attention_block(bi):
            jmax = 4 * bi + 3
            npair = (jmax + 1) // 2

            for h in range(HPG):
                acc = P2.tile([128, 2 * TB], F32, tag="acc", name="acc")
                pso = acc[:, 0:TB]      # PV accumulator   (bank A of slot)
                psd = acc[:, TB:2 * TB]  # denominator     (bank B of slot)

                def spair(p):
                    """Two adjacent s-tile score matmuls into one 2-bank tile."""
                    pss = P1.tile([128, 2 * TB], F32, tag="ps", name="pss")
                    info = []
                    for k2 in (0, 1):
                        j = 2 * p + k2
                        o = 128 * (j - 4 * bi) if (j // 4 == bi) else 0
                        nc.tensor.matmul(
                            pss[:, k2 * TB + o:(k2 + 1) * TB],
                            krot[:, j * 128:(j + 1) * 128],
                            qrot[h][:, bi * TB + o:(bi + 1) * TB],
                            start=True, stop=True)
                        info.append((j, o))
                    return pss, info

                cur = spair(0)
                if pend:
                    flush_pend()   # last rope part-2 rides under h=0 scores
                for p in range(npair):
                    nxt = spair(p + 1) if p + 1 < npair else None
                    pss, info = cur
                    o0 = info[0][1]
                    pt = ptpool.tile([128, 2 * TB], BF16, tag="pt", name="pt")
                    # one exp over both banks (garbage strip between diag
                    # halves is never read downstream)
                    nc.scalar.activation(pt[:, o0:2 * TB], pss[:, o0:2 * TB], EXP)
                    for k2, (j, o) in enumerate(info):
                        kb = k2 * TB
                        if j // 4 == bi:
                            nc.vector.tensor_mul(pt[:, kb + o:kb + o + 128],
                                                 pt[:, kb + o:kb + o + 128], tri[:])
                        nc.tensor.matmul(psd[:, o:TB], ones[:], pt[:, kb + o:kb + TB],
                                         start=(j == 0), stop=(j == jmax))
                        nc.tensor.matmul(pso[:, o:TB], vsd[:, j * 128:(j + 1) * 128],
                                         pt[:, kb + o:kb + TB],
                                         start=(j == 0), stop=(j == jmax))
                    cur = nxt
                cols = slice(bi * TB, (bi + 1) * TB)
                rden = srpool.tile([128, TB], F32, tag="rden", name="rden")
                nc.vector.reciprocal_approx_fast(rden[:], psd)
                nc.vector.tensor_mul(aout[h][:, cols], pso, rden[:])

        # ======== fused projection + attention (per 512-token block) ========
        for tb in range(NTB):
            project_block(tb)
            if tb == 1:
                nc.scalar.dma_start(wo_t[:], ins["wo"][:])
            attention_block(tb)

        # ================= output projection =================
        for tq in range(NST):
            trows = slice(tq * 128, (tq + 1) * 128)
            acc1 = P1.tile([128, 2 * TB], F32, tag="ps", name="po01")
            acc2 = P2.tile([128, 2 * TB], F32, tag="acc", name="po23")
            pos = [acc1[:, 0:TB], acc1[:, TB:2 * TB],
                   acc2[:, 0:TB], acc2[:, TB:2 * TB]]
            for hh in range(HPG):
                lh = aout[hh][:, trows]
                for eo in range(4):
                    nc.tensor.matmul(pos[eo],
                                     lh, wo_t[:, (hh * 4 + eo) * 512:(hh * 4 + eo + 1) * 512],
                                     start=(hh == 0), stop=(hh == HPG - 1))
            osb = ospool.tile([128, 4 * TB], BF16, tag="osb", name="osb")
            for eo in range(4):
                eng = nc.vector.tensor_copy if eo % 2 == 0 else nc.scalar.copy
                eng(osb[:, eo * TB:(eo + 1) * TB], pos[eo])
            nc.sync.dma_start(out_ap[trows, :], osb[:])


# ---------------- host side ----------------

_PERM = np.concatenate([np.arange(0, HD, 2), np.arange(1, HD, 2)])  # half-split


def _chunked(a, ncols):
    """[E, ncols] -> [128, NE*ncols] with col block e = rows e*128:(e+1)*128."""
    return np.ascontiguousarray(
        a.reshape(NE, 128, ncols).transpose(1, 0, 2).reshape(128, NE * ncols))


def host_prep(inputs):
    """Full inputs -> list of 8 per-core input dicts (core i = (b=i//4, g=i%4))."""
    x = np.asarray(inputs["x"], dtype=np.float32)
    Wq = np.asarray(inputs["Wq"], dtype=np.float32)
    Wk = np.asarray(inputs["Wk"], dtype=np.float32)
    Wv = np.asarray(inputs["Wv"], dtype=np.float32)
    Wo = np.asarray(inputs["Wo"], dtype=np.float32)

    inv = (10000.0 ** (-np.arange(0, HD, 2, dtype=np.float32) / HD)).astype(np.float32)
    tpos = np.arange(T, dtype=np.float32)
    fr = np.outer(tpos, inv)                       # [T, 64]
    cosT = np.cos(fr).T.astype(np.float32)         # [64, T]
    sinT = np.sin(fr).T.astype(np.float32)
    cs2 = np.concatenate([cosT, cosT], axis=0).astype(BF)     # [128, T]
    snpm = np.concatenate([-sinT, sinT], axis=0).astype(BF)   # [128, T]

    tri = (np.arange(128)[None, :] >= np.arange(128)[:, None]).astype(BF)
    swp = np.zeros((128, 128), dtype=np.float32)
    swp[(np.arange(128) + 64) % 128, np.arange(128)] = 1.0
    swp = swp.astype(BF)
    iden = np.eye(128, dtype=np.float32).astype(BF)

    scale = np.float32(1.0 / np.sqrt(HD))
    # x[b].T chunked: xh[p, e*T + t] = x[b][t, e*128+p]
    xh = [_chunked(np.ascontiguousarray(x[b].T), T).astype(BF) for b in range(B)]

    in_maps = []
    for i in range(N_CORES):
        b, g = i // 4, i % 4
        # wq: rows for heads g*4..g*4+3, each permuted, scaled; -> [E, 512]
        rows = []
        for h in range(HPG):
            base = (g * HPG + h) * HD
            rows.append(Wq[base + _PERM, :])
        wq_c = (np.concatenate(rows, axis=0) * scale).T  # [E, 512]
        wk_c = Wk[g * HD + _PERM, :].T                   # [E, 128]
        wv_c = Wv[g * HD:(g + 1) * HD, :].T              # [E, 128]
        wo_c = np.ascontiguousarray(Wo[:, g * 512:(g + 1) * 512].T)  # [512, E]
        # device layout [128, 16*512]: col block (hh*4+eo) = wo_c[hh*128:.., eo*512:..]
        wo_p = np.ascontiguousarray(
            wo_c.reshape(HPG, 128, 4, 512).transpose(1, 0, 2, 3).reshape(128, NE * 512))
        in_maps.append({
            "xh": xh[b],
            "wq": _chunked(wq_c, 512).astype(BF),
            "wk": _chunked(wk_c, 128).astype(BF),
            "wv": _chunked(wv_c, 128).astype(BF),
            "wo": wo_p.astype(BF),
            "cs2": cs2, "snpm": snpm, "tri": tri, "swp": swp, "iden": iden,
            "onec": np.ones((128, 128), dtype=BF),
        })
    return in_maps


_NC = None


def build_nc():
    global _NC
    if _NC is not None:
        return _NC
    nc = bacc.Bacc("TRN2", target_bir_lowering=False, debug=False,
                   num_devices=N_CORES)
    ins = {
        "xh": nc.dram_tensor("xh", [128, NE * T], BF16, kind="ExternalInput").ap(),
        "wq": nc.dram_tensor("wq", [128, NE * 512], BF16, kind="ExternalInput").ap(),
        "wk": nc.dram_tensor("wk", [128, NE * 128], BF16, kind="ExternalInput").ap(),
        "wv": nc.dram_tensor("wv", [128, NE * 128], BF16, kind="ExternalInput").ap(),
        "wo": nc.dram_tensor("wo", [128, NE * 512], BF16, kind="ExternalInput").ap(),
        "cs2": nc.dram_tensor("cs2", [128, T], BF16, kind="ExternalInput").ap(),
        "snpm": nc.dram_tensor("snpm", [128, T], BF16, kind="ExternalInput").ap(),
        "tri": nc.dram_tensor("tri", [128, 128], BF16, kind="ExternalInput").ap(),
        "swp": nc.dram_tensor("swp", [128, 128], BF16, kind="ExternalInput").ap(),
        "iden": nc.dram_tensor("iden", [128, 128], BF16, kind="ExternalInput").ap(),
        "onec": nc.dram_tensor("onec", [128, 128], BF16, kind="ExternalInput").ap(),
    }
    out = nc.dram_tensor("out", [T, E], BF16, kind="ExternalOutput").ap()
    with tile.TileContext(nc) as tc:
        build_body(tc, out, ins)
    nc.compile()
    _NC = nc
    return nc


def gather(results):
    """results: list of 8 dicts with 'out' [T, E] bf16 partials -> [B, T, E] f32."""
    out = np.zeros((B, T, E), dtype=np.float32)
    for i in range(N_CORES):
        out[i // 4] += np.asarray(results[i]["out"]).astype(np.float32)
    return out


def kernel(**inputs):
    nc = build_nc()
    in_maps = host_prep(inputs)
    res = run_bass_kernel_spmd(nc, in_maps, core_ids=list(range(N_CORES)))
    return gather(res.results)


if __name__ == "__main__":
    rng = np.random.default_rng(0)
    ins = {
        "x": rng.standard_normal((B, T, E), dtype=np.float32),
        "Wq": rng.standard_normal((E, E), dtype=np.float32) * 0.02,
        "Wk": rng.standard_normal((NG * HD, E), dtype=np.float32) * 0.02,
        "Wv": rng.standard_normal((NG * HD, E), dtype=np.float32) * 0.02,
        "Wo": rng.standard_normal((E, E), dtype=np.float32) * 0.02,
    }
    out = kernel(**ins)
    print(out.shape, out.dtype, np.abs(out).mean())
